# revision 1
# baseline (speedup 1.0000x reference)
"""DeepFRI GCN (3x GraphConv + mean-pool + MLP head) on 8 Trainium2 NeuronCores.

Strategy:
  - Renumber nodes into 80 tiles of <=128 slots (bin-packed to balance
    in-edge counts); tiles t in [c*10, (c+1)*10) live on core c.
  - Aggregation (segment_sum with both-side norms) is a one-hot matmul on
    the PE: gather x[src] rows with dma_gather (edges sorted per dst tile,
    128 edges per block), then psum += S_b^T @ G_b where
    S_b[lane, dstslot] = out_isqrt[src] * in_isqrt[dst].
  - Layer 1 computes full per-node f1 (needed for layer-2 gathers):
    transpose agg1, dense matmul with W0, + b0; AllGather f1.
  - The output only needs column-sums of f1/f2/f3 (mean-pool), so:
      colsum1 = (1^T agg1) W0 + n b0
      colsum2 = (1^T agg2) W1 + n b1
      colsum3 = (w^T agg2) W1 W2 + (sum w) b1 W2 + n b2,
        where w[u] = out_isqrt[u] * sum_{e:src=u} in_isqrt[dst_e]
    i.e. layer 2 needs no dense matmul and layer 3 needs no aggregation.
  - AllReduce the 3x1280 partial colsums, then every core computes the
    tiny MLP head redundantly.
"""

import math
import numpy as np

P = 128
N_CORES = 8
TPC = 10                    # tiles per core
N_TILES = N_CORES * TPC     # 80
D = 1280
NKI = D // P                # 10 k-chunks of 128
OC_SIZES = (512, 512, 256)  # output/free chunking of 1280
OC_OFFS = (0, 512, 1024)


# ---------------------------------------------------------------------------
# host-side graph preprocessing (pure index/layout work + degree norms)
# ---------------------------------------------------------------------------

def preprocess(edge_index, n_nodes):
    import heapq

    src = np.asarray(edge_index[0], dtype=np.int64)
    dst = np.asarray(edge_index[1], dtype=np.int64)
    out_deg = np.bincount(src, minlength=n_nodes).astype(np.float64)
    in_deg = np.bincount(dst, minlength=n_nodes).astype(np.float64)
    out_isqrt = (1.0 / np.sqrt(np.clip(out_deg, 1.0, None))).astype(np.float32)
    in_isqrt = (1.0 / np.sqrt(np.clip(in_deg, 1.0, None))).astype(np.float32)
    # w[u] = out_isqrt[u] * sum_{e: src=u} in_isqrt[dst_e]
    w = out_isqrt * np.bincount(
        src, weights=in_isqrt[dst].astype(np.float64), minlength=n_nodes
    ).astype(np.float32)
    sigma_w = float(w.astype(np.float64).sum())

    # --- bin-pack nodes into N_TILES tiles (<=P nodes each), balancing
    # in-edge loads so every tile needs the same number of edge blocks.
    in_deg_i = in_deg.astype(np.int64)
    order = np.argsort(-in_deg_i, kind="stable")
    tile_of = np.empty(n_nodes, np.int32)
    slot_of = np.empty(n_nodes, np.int32)
    counts = np.zeros(N_TILES, np.int32)
    heap = [(0, t) for t in range(N_TILES)]
    heapq.heapify(heap)
    spill = []
    for node in order:
        while True:
            load, t = heapq.heappop(heap)
            if counts[t] < P:
                break
            spill.append((load, t))
        tile_of[node] = t
        slot_of[node] = counts[t]
        counts[t] += 1
        heapq.heappush(heap, (load + int(in_deg_i[node]), t))
        for it in spill:
            heapq.heappush(heap, it)
        spill.clear()
    padded_id = tile_of.astype(np.int64) * P + slot_of.astype(np.int64)

    # --- per-tile edge lists (edges grouped by dst tile)
    etile = tile_of[dst]
    eorder = np.argsort(etile, kind="stable")
    e_sorted = eorder
    tile_edge_counts = np.bincount(etile, minlength=N_TILES)
    B = max(1, int(math.ceil(tile_edge_counts.max() / P)))
    SLOTS = B * P  # edge slots per tile

    nslot = N_TILES * SLOTS
    gsrc_orig = np.zeros(nslot, np.int64)
    gsrc_pad = np.zeros(nslot, np.int64)
    sval = np.zeros(nslot, np.float32)
    dcol = np.zeros(nslot, np.int64)

    starts = np.zeros(N_TILES + 1, np.int64)
    np.cumsum(tile_edge_counts, out=starts[1:])
    for t in range(N_TILES):
        es = e_sorted[starts[t] : starts[t + 1]]
        k = es.shape[0]
        base = t * SLOTS
        gsrc_orig[base : base + k] = src[es]
        gsrc_pad[base : base + k] = padded_id[src[es]]
        sval[base : base + k] = out_isqrt[src[es]] * in_isqrt[dst[es]]
        dcol[base : base + k] = slot_of[dst[es]]

    # --- S matrices, DRAM layout [128(lane), N_TILES*B*128(tile,block,dstcol)]
    # S[lane, (t*B+b)*128 + col] = sval of edge slot (t, b*128+lane) if its
    # dst col == col else 0
    s_mat = np.zeros((N_TILES, B, P, P), np.float32)  # [t, b, lane, col]
    sl = np.arange(nslot)
    t_i = sl // SLOTS
    b_i = (sl % SLOTS) // P
    lane_i = sl % P
    s_mat[t_i, b_i, lane_i, dcol] = sval
    # per-core [128, TPC*B*128]
    s_core = (
        s_mat.reshape(N_CORES, TPC * B, P, P)
        .transpose(0, 2, 1, 3)
        .reshape(N_CORES, P, TPC * B * P)
    )
    s_core = np.ascontiguousarray(s_core)

    # --- gather index tiles: int16, 16-partition wrap, replicated to 128
    def wrap_idx(idx_flat):
        # idx_flat [n] with n % 16 == 0 -> [128, n//16], unwrap order
        # unwrapped[i] = tile[i % 16, i // 16]
        n = idx_flat.shape[0]
        a = idx_flat.reshape(n // 16, 16).T.astype(np.int16)  # [16, n//16]
        return np.ascontiguousarray(np.tile(a, (8, 1)))  # [128, n//16]

    idx1 = np.stack(
        [wrap_idx(gsrc_orig.reshape(N_CORES, -1)[c]) for c in range(N_CORES)]
    )
    idx2 = np.stack(
        [wrap_idx(gsrc_pad.reshape(N_CORES, -1)[c]) for c in range(N_CORES)]
    )

    # --- uw: [core][128, TPC, 2]: col0 = valid-ones, col1 = w at the slot
    uw = np.zeros((N_TILES, P, 2), np.float32)
    uw[tile_of, slot_of, 0] = 1.0
    uw[tile_of, slot_of, 1] = w
    uw_core = np.ascontiguousarray(
        uw.reshape(N_CORES, TPC, P, 2).transpose(0, 2, 1, 3)
    )  # [core, 128, TPC, 2]

    return dict(
        B=B,
        tile_of=tile_of,
        slot_of=slot_of,
        padded_id=padded_id,
        gsrc_orig=gsrc_orig,
        gsrc_pad=gsrc_pad,
        sval=sval,
        dcol=dcol,
        s_core=s_core,
        idx1=idx1,
        idx2=idx2,
        uw_core=uw_core,
        sigma_w=sigma_w,
        out_isqrt=out_isqrt,
        in_isqrt=in_isqrt,
        w=w,
    )


def bcat_T(b0, b1, b2):
    # [128, 30] column layout of concat([b0,b1,b2]): [:, l*10+c][p] = b_l[c*128+p]
    cols = []
    for b in (b0, b1, b2):
        cols.append(np.asarray(b, np.float32).reshape(NKI, P).T)  # [128, 10]
    return np.ascontiguousarray(np.concatenate(cols, axis=1))


# ---------------------------------------------------------------------------
# numpy golden model of the exact device algorithm (for validation)
# ---------------------------------------------------------------------------

def golden(node_feat0, edge_index, n_nodes, W0, b0, W1, b1, W2, b2,
           Wh1, bh1, Wh2, bh2, pre=None):
    x = np.asarray(node_feat0, np.float32)
    n_nodes = int(n_nodes)
    if pre is None:
        pre = preprocess(edge_index, n_nodes)
    B = pre["B"]
    SLOTS = B * P
    s_mat = np.zeros((N_TILES, B, P, P), np.float32)
    sl = np.arange(N_TILES * SLOTS)
    s_mat[sl // SLOTS, (sl % SLOTS) // P, sl % P, pre["dcol"]] = pre["sval"]

    # layer 1
    G1 = x[pre["gsrc_orig"]].reshape(N_TILES, B, P, D)
    agg1 = np.einsum("tblc,tblf->tcf", s_mat, G1)  # [tile, slot, feat]
    f1 = agg1 @ np.asarray(W0, np.float32) + np.asarray(b0, np.float32)
    aggcol1 = agg1.sum(axis=(0, 1))  # [D]
    r1 = aggcol1 @ np.asarray(W0, np.float32)

    f1_full = f1.reshape(N_TILES * P, D)
    # layer 2
    G2 = f1_full[pre["gsrc_pad"]].reshape(N_TILES, B, P, D)
    agg2 = np.einsum("tblc,tblf->tcf", s_mat, G2)
    aggcol2 = agg2.sum(axis=(0, 1))
    r2 = aggcol2 @ np.asarray(W1, np.float32)
    # layer 3 (collapsed)
    uw = np.zeros((N_TILES, P), np.float32)
    uw[pre["tile_of"], pre["slot_of"]] = pre["w"]
    wagg2 = np.einsum("tc,tcf->f", uw, agg2)
    wf2 = wagg2 @ np.asarray(W1, np.float32) + pre["sigma_w"] * np.asarray(b1, np.float32)
    r3 = wf2 @ np.asarray(W2, np.float32)

    g = np.concatenate([r1, r2, r3]) / n_nodes + np.concatenate(
        [np.asarray(b0, np.float32), np.asarray(b1, np.float32), np.asarray(b2, np.float32)]
    )
    h = np.maximum(g, 0.0) @ np.asarray(Wh1, np.float32) + np.asarray(bh1, np.float32)
    h = np.maximum(h, 0.0) @ np.asarray(Wh2, np.float32) + np.asarray(bh2, np.float32)
    return np.tanh(h)


# ---------------------------------------------------------------------------
# Bass kernel
# ---------------------------------------------------------------------------

def build_nc(pre, n_nodes, sigma_w, repeat=1, stage="full"):
    """Build the (SPMD, identical across cores) Bass program."""
    import concourse.bacc as bacc
    import concourse.mybir as mybir
    import concourse.tile as tile

    f32 = mybir.dt.float32
    f32r = mybir.dt.float32r
    i16 = mybir.dt.int16
    AX = mybir.AxisListType.X
    RELU = mybir.ActivationFunctionType.Relu
    TANH = mybir.ActivationFunctionType.Tanh

    B = pre["B"]
    # gather chunking: chunks of <=3 blocks
    gch = []
    b0_ = 0
    while b0_ < B:
        gch.append((b0_, min(2, B - b0_)))
        b0_ += min(2, B - b0_)

    nc = bacc.Bacc(
        "TRN2",
        target_bir_lowering=False,
        debug=False,
        num_devices=N_CORES,
        num_swdge_queues=4,
        dynamic_dma_scratch_size=65536,
    )

    # ---- kernel I/O
    x_d = nc.dram_tensor("x", [n_nodes, D], f32r, kind="ExternalInput")
    w0_d = nc.dram_tensor("w0", [D, D], f32r, kind="ExternalInput")
    w1_d = nc.dram_tensor("w1", [D, D], f32r, kind="ExternalInput")
    w2_d = nc.dram_tensor("w2", [D, D], f32r, kind="ExternalInput")
    wh1_d = nc.dram_tensor("wh1", [3 * D, P], f32r, kind="ExternalInput")
    wh2_d = nc.dram_tensor("wh2", [P, 2], f32r, kind="ExternalInput")
    b0b_d = nc.dram_tensor("b0b", [P, D], f32, kind="ExternalInput")
    b1r_d = nc.dram_tensor("b1r", [1, D], f32, kind="ExternalInput")
    bcat_d = nc.dram_tensor("bcat", [P, 3 * NKI], f32, kind="ExternalInput")
    bh1_d = nc.dram_tensor("bh1", [1, P], f32, kind="ExternalInput")
    bh2_d = nc.dram_tensor("bh2", [1, 2], f32, kind="ExternalInput")
    ident_d = nc.dram_tensor("ident", [P, P], f32r, kind="ExternalInput")
    smat_d = nc.dram_tensor("smat", [P, TPC * B * P], f32r, kind="ExternalInput")
    idx1_d = nc.dram_tensor("idx1", [P, TPC * B * P // 16], i16, kind="ExternalInput")
    idx2_d = nc.dram_tensor("idx2", [P, TPC * B * P // 16], i16, kind="ExternalInput")
    uw_d = nc.dram_tensor("uw", [P, TPC * 2], f32r, kind="ExternalInput")
    out_d = nc.dram_tensor("out", [1, 2], f32, kind="ExternalOutput")

    with tile.TileContext(nc) as tc:
        with (
            nc.allow_low_precision(reason="fp32r operand chain feeding PE"),
            tc.tile_pool(name="dram", bufs=1, space="DRAM") as dram,
            tc.tile_pool(name="const", bufs=1) as const,
            tc.tile_pool(name="wpool", bufs=1) as wpool,
            tc.tile_pool(name="gpool", bufs=2) as gpool,
            tc.tile_pool(name="work", bufs=2) as work,
            tc.tile_pool(name="fpool", bufs=2) as fpool,
            tc.tile_pool(name="accp", bufs=1) as accp,
            tc.tile_pool(name="psA", bufs=1, space="PSUM") as psA,
            tc.tile_pool(name="psT", bufs=2, space="PSUM") as psT,
            tc.tile_pool(name="psF", bufs=1, space="PSUM") as psF,
        ):
            # ---- load constants
            s_sb = const.tile([P, TPC * B * P], f32r, name="s_sb")
            nc.sync.dma_start(s_sb[:], smat_d[:, :])
            idx1_sb = const.tile([P, TPC * B * P // 16], i16, name="idx1_sb")
            nc.sync.dma_start(idx1_sb[:], idx1_d[:, :])
            idx2_sb = const.tile([P, TPC * B * P // 16], i16, name="idx2_sb")
            nc.sync.dma_start(idx2_sb[:], idx2_d[:, :])
            uw_sb = const.tile([P, TPC, 2], f32r, name="uw_sb")
            nc.sync.dma_start(uw_sb[:], uw_d[:, :].rearrange("p (t c) -> p t c", c=2))
            ident_sb = const.tile([P, P], f32r, name="ident_sb")
            nc.sync.dma_start(ident_sb[:], ident_d[:, :])
            b0b_sb = const.tile([P, D], f32, name="b0b_sb")
            nc.sync.dma_start(b0b_sb[:], b0b_d[:, :])
            b1r_sb = const.tile([1, D], f32, name="b1r_sb")
            nc.sync.dma_start(b1r_sb[:], b1r_d[:, :])
            bcat_sb = const.tile([P, 3 * NKI], f32, name="bcat_sb")
            nc.sync.dma_start(bcat_sb[:], bcat_d[:, :])
            bh1_sb = const.tile([1, P], f32, name="bh1_sb")
            nc.sync.dma_start(bh1_sb[:], bh1_d[:, :])
            bh2_sb = const.tile([1, 2], f32, name="bh2_sb")
            nc.sync.dma_start(bh2_sb[:], bh2_d[:, :])
            wh2_sb = const.tile([P, 2], f32r, name="wh2_sb")
            nc.sync.dma_start(wh2_sb[:], wh2_d[:, :])

            def pipeline():
              # ---- internal DRAM (per repeat; fresh allocations)
              f1_slice = dram.tile([TPC * P, D], f32r, name="f1_slice", tag="f1s")
              f1_full = dram.tile([N_TILES * P, D], f32r, name="f1_full",
                                  addr_space="Shared", tag="f1f")
              crt_rows_d = dram.tile([3, D], f32, name="crt_rows_d", tag="crd")
              crt_tot_d = dram.tile([3, D], f32, name="crt_tot_d",
                                    addr_space="Shared", tag="ctd")
              rowb_d = dram.tile([2, D], f32r, name="rowb_d", tag="rbd")
              wfr_d = dram.tile([1, D], f32r, name="wfr_d", tag="wfd")
              hb_d = dram.tile([1, P], f32r, name="hb_d", tag="hbd")

              # accumulators
              aggcol1T = accp.tile([P, NKI], f32r, name="aggcol1T", tag="aggcol1T")
              nc.vector.memset(aggcol1T[:].bitcast(f32), 0.0)
              acc2 = accp.tile([2, D], f32r, name="acc2", tag="acc2")
              nc.vector.memset(acc2[:, :].bitcast(f32), 0.0)
              r1_sb = accp.tile([1, D], f32, name="r1_sb", tag="rrow")

              def load_w(wd, name):
                  wt = wpool.tile([P, NKI, D], f32r, name=name, tag="w")
                  nc.sync.dma_start(
                      wt[:], wd[:, :].rearrange("(k p) j -> p k j", p=P)
                  )
                  return wt

              w_sb = load_w(w0_d, "w0_sb")

              def gathers(t, idx_sb, src_d, layer):
                  tiles = []
                  for ci, (cb, nb) in enumerate(gch):
                      g_t = gpool.tile([P, nb, D], f32r, name=f"g_{layer}_{t}_{ci}",
                                       tag="g")
                      off16 = (t * B + cb) * P // 16
                      n16 = nb * P // 16
                      nc.gpsimd.dma_gather(
                          out_ap=g_t[:, :, :],
                          in_ap=src_d[:, :],
                          idxs_ap=idx_sb[:, off16 : off16 + n16],
                          num_idxs=nb * P,
                          num_idxs_reg=nb * P,
                          elem_size=D,
                          queue_num=(t * len(gch) + ci) % 4,
                      )
                      tiles.append((cb, nb, g_t))
                  return tiles

              def agg_layer(t, gtiles):
                  """one-hot matmuls -> node-major agg psum [128, 1280] (3 tiles)"""
                  agg_ps = [
                      psA.tile([P, oc], f32, name=f"agg_ps{i}_{t}", tag=f"agg{i}")
                      for i, oc in enumerate(OC_SIZES)
                  ]
                  for cb, nb, g_t in gtiles:
                      for bb in range(nb):
                          b = cb + bb
                          s_ap = s_sb[:, (t * B + b) * P : (t * B + b + 1) * P]
                          for i, (oc, off) in enumerate(zip(OC_SIZES, OC_OFFS)):
                              nc.tensor.matmul(
                                  agg_ps[i][:, :],
                                  s_ap,
                                  g_t[:, bb, off : off + oc],
                                  start=(b == 0),
                                  stop=(b == B - 1),
                              )
                  # copy psum -> sbuf (ACT engine, keeps DVE free)
                  agg_sb = work.tile([P, D], f32r, name=f"agg_sb_{t}", tag="aggsb")
                  for i, (oc, off) in enumerate(zip(OC_SIZES, OC_OFFS)):
                      nc.scalar.copy(agg_sb[:, off : off + oc], agg_ps[i][:, :])
                  return agg_sb

              # ================= LAYER 1 =================
              for t in range(TPC):
                  gtiles = gathers(t, idx1_sb, x_d, 1)
                  agg_sb = agg_layer(t, gtiles)

                  # transpose agg -> aggT  (PE transpose, 4 chunks per psum tile)
                  aggT_sb = work.tile([P, NKI * P], f32r, name=f"aggT_sb_{t}",
                                      tag="aggT")
                  for grp0 in range(0, NKI, 4):
                      ng = min(4, NKI - grp0)
                      tp_ps = psT.tile([P, ng * P], f32r, name=f"tp_{t}_{grp0}",
                                       tag="tp")
                      for j in range(ng):
                          ki = grp0 + j
                          nc.tensor.transpose(
                              tp_ps[:, j * P : (j + 1) * P],
                              agg_sb[:, ki * P : (ki + 1) * P],
                              ident_sb[:, :],
                          )
                      nc.scalar.copy(
                          aggT_sb[:, grp0 * P : (grp0 + ng) * P], tp_ps[:, :]
                      )

                  # column-sum of agg (for colsum1): reduce over dst slots
                  red = work.tile([P, NKI], f32r, name=f"red_{t}", tag="red")
                  nc.vector.reduce_sum(
                      red[:],
                      aggT_sb[:].rearrange("p (k d) -> p k d", d=P),
                      axis=AX,
                  )
                  nc.vector.tensor_add(aggcol1T[:], aggcol1T[:], red[:])

                  # dense: f1 = agg @ W0  (accumulate over NKI k-chunks)
                  f_ps = [
                      psF.tile([P, oc], f32, name=f"f_ps{i}_{t}", tag=f"f{i}")
                      for i, oc in enumerate(OC_SIZES)
                  ]
                  for i, (oc, off) in enumerate(zip(OC_SIZES, OC_OFFS)):
                      for ki in range(NKI):
                          nc.tensor.matmul(
                              f_ps[i][:, :],
                              aggT_sb[:, ki * P : (ki + 1) * P],
                              w_sb[:, ki, off : off + oc],
                              start=(ki == 0),
                              stop=(ki == NKI - 1),
                          )
                  f_sb = fpool.tile([P, D], f32r, name=f"f_sb_{t}", tag="fsb")
                  for i, (oc, off) in enumerate(zip(OC_SIZES, OC_OFFS)):
                      nc.vector.tensor_add(
                          f_sb[:, off : off + oc],
                          f_ps[i][:, :],
                          b0b_sb[:, off : off + oc],
                      )
                  nc.sync.dma_start(f1_slice[t * P : (t + 1) * P, :], f_sb[:])

              # r1 = aggcol1 @ W0  -> crt_rows[0]
              def matvec(colT_ap_of_ki, wt, out_row_ap, row_tag):
                  for i, (oc, off) in enumerate(zip(OC_SIZES, OC_OFFS)):
                      r_ps = psT.tile([1, oc], f32, name=f"r_{row_tag}_{i}",
                                      tag="tp")
                      for ki in range(NKI):
                          nc.tensor.matmul(
                              r_ps[:, :],
                              colT_ap_of_ki(ki),
                              wt[:, ki, off : off + oc],
                              start=(ki == 0),
                              stop=(ki == NKI - 1),
                          )
                      nc.vector.tensor_copy(out_row_ap[:, off : off + oc], r_ps[:, :])

              matvec(lambda ki: aggcol1T[:, ki : ki + 1], w_sb,
                     r1_sb[:, :], "r1")

              if stage == "l1":
                  nc.sync.dma_start(out_d[:, :], r1_sb[0:1, 0:2])
                  return

              # AllGather f1
              nc.gpsimd.collective_compute(
                  "AllGather",
                  mybir.AluOpType.bypass,
                  replica_groups=[list(range(N_CORES))],
                  ins=[f1_slice[:, :]],
                  outs=[f1_full[:, :]],
              )

              if stage == "l1ag":
                  nc.sync.dma_start(out_d[:, :], r1_sb[0:1, 0:2])
                  return

              # ================= LAYER 2 =================
              w_sb = load_w(w1_d, "w1_sb")
              for t in range(TPC):
                  gtiles = gathers(t, idx2_sb, f1_full, 2)
                  agg_sb = agg_layer(t, gtiles)
                  # rows: [ones; w]^T @ agg  -> acc2 [2, 1280]
                  for i, (oc, off) in enumerate(zip(OC_SIZES, OC_OFFS)):
                      row_ps = psT.tile([2, oc], f32, name=f"row2_{t}_{i}",
                                        tag="tp")
                      nc.tensor.matmul(
                          row_ps[:, :],
                          uw_sb[:, t, :],
                          agg_sb[:, off : off + oc],
                          start=True,
                          stop=True,
                      )
                      nc.vector.tensor_add(
                          acc2[:, off : off + oc], acc2[:, off : off + oc],
                          row_ps[:, :],
                      )

              if stage == "l2":
                  nc.sync.dma_start(out_d[:, :], acc2[0:1, 0:2].bitcast(f32))
                  return

              # transpose acc2 rows -> columns via DRAM bounce
              nc.sync.dma_start(rowb_d[:, :], acc2[:, :])
              acc2T = accp.tile([P, 2, NKI], f32r, name="acc2T", tag="acc2T")
              nc.sync.dma_start(
                  acc2T[:],
                  rowb_d[:, :].rearrange("j (k p) -> p j k", p=P),
              )
              # r2 = aggcol2 @ W1 -> crt_rows[1]
              r2_sb = accp.tile([1, D], f32, name="r2_sb", tag="rrow")
              matvec(lambda ki: acc2T[:, 0, ki : ki + 1], w_sb,
                     r2_sb[:, :], "r2")
              # wf2 = wagg2 @ W1 + sigma_w * b1
              wf_row = accp.tile([1, D], f32r, name="wf_row", tag="rrow")
              for i, (oc, off) in enumerate(zip(OC_SIZES, OC_OFFS)):
                  r_ps = psT.tile([1, oc], f32, name=f"r_wf_{i}", tag="tp")
                  for ki in range(NKI):
                      nc.tensor.matmul(
                          r_ps[:, :],
                          acc2T[:, 1, ki : ki + 1],
                          w_sb[:, ki, off : off + oc],
                          start=(ki == 0),
                          stop=(ki == NKI - 1),
                      )
                  nc.vector.scalar_tensor_tensor(
                      wf_row[:, off : off + oc],
                      b1r_sb[:, off : off + oc],
                      float(sigma_w),
                      r_ps[:, :],
                      mybir.AluOpType.mult,
                      mybir.AluOpType.add,
                  )
              nc.sync.dma_start(wfr_d[:, :], wf_row[:, :])
              wf2T = accp.tile([P, NKI], f32r, name="wf2T", tag="wf2T")
              nc.sync.dma_start(
                  wf2T[:], wfr_d[0, :].rearrange("(k p) -> p k", p=P)
              )
              # r3 = wf2 @ W2 -> crt_rows[2]
              w_sb = load_w(w2_d, "w2_sb")
              r3_sb = accp.tile([1, D], f32, name="r3_sb", tag="rrow")
              matvec(lambda ki: wf2T[:, ki : ki + 1], w_sb,
                     r3_sb[:, :], "r3")

              # ---- AllReduce partial colsums
              nc.sync.dma_start(crt_rows_d[0:1, :], r1_sb[:, :])
              nc.sync.dma_start(crt_rows_d[1:2, :], r2_sb[:, :])
              nc.sync.dma_start(crt_rows_d[2:3, :], r3_sb[:, :])
              nc.gpsimd.collective_compute(
                  "AllReduce",
                  mybir.AluOpType.add,
                  replica_groups=[list(range(N_CORES))],
                  ins=[crt_rows_d[:, :]],
                  outs=[crt_tot_d[:, :]],
              )
              ctT = accp.tile([P, 3, NKI], f32, name="ctT", tag="ctT")
              nc.sync.dma_start(
                  ctT[:],
                  crt_tot_d[0:3, :].rearrange("l (k p) -> p l k", p=P),
              )

              # ---- head (redundant on every core)
              wh1_sb = wpool.tile([P, 3 * NKI, P], f32r, name="wh1_sb", tag="w")
              nc.sync.dma_start(
                  wh1_sb[:],
                  wh1_d[:, :].rearrange("(c p) j -> p c j", p=P),
              )
              gr = accp.tile([P, 3 * NKI], f32, name="gr", tag="gr")
              nc.vector.scalar_tensor_tensor(
                  gr[:],
                  ctT[:].rearrange("p l k -> p (l k)"),
                  1.0 / float(n_nodes),
                  bcat_sb[:],
                  mybir.AluOpType.mult,
                  mybir.AluOpType.add,
              )
              grr = accp.tile([P, 3 * NKI], f32r, name="grr", tag="grr")
              nc.scalar.activation(grr[:], gr[:], RELU)

              h1_ps = psT.tile([1, P], f32, name="h1_ps", tag="tp")
              for c in range(3 * NKI):
                  nc.tensor.matmul(
                      h1_ps[:, :],
                      grr[:, c : c + 1],
                      wh1_sb[:, c, :],
                      start=(c == 0),
                      stop=(c == 3 * NKI - 1),
                  )
              h1r = accp.tile([1, P], f32r, name="h1r", tag="h1r")
              nc.vector.tensor_add(h1r[:], h1_ps[:, :], bh1_sb[:])
              nc.scalar.activation(h1r[:], h1r[:], RELU)

              nc.sync.dma_start(hb_d[:, :], h1r[:, :])
              h1T = accp.tile([P, 1], f32r, name="h1T", tag="h1T")
              nc.sync.dma_start(h1T[:], hb_d[0, :].rearrange("(p j) -> p j", j=1))

              o_ps = psT.tile([1, 2], f32, name="o_ps", tag="tp")
              nc.tensor.matmul(
                  o_ps[:, :], h1T[:], wh2_sb[:, :],
                  start=True, stop=True,
              )
              o_sb = accp.tile([1, 2], f32, name="o_sb", tag="o_sb")
              nc.vector.tensor_add(o_sb[:], o_ps[:, :], bh2_sb[:])
              nc.scalar.activation(o_sb[:], o_sb[:], TANH)
              nc.sync.dma_start(out_d[:, :], o_sb[:])

            for _rep in range(repeat):
                pipeline()

    nc.compile()
    return nc


def make_in_maps(inputs, pre):
    x = np.ascontiguousarray(np.asarray(inputs["node_feat0"], np.float32))
    W0 = np.ascontiguousarray(np.asarray(inputs["W0"], np.float32))
    W1 = np.ascontiguousarray(np.asarray(inputs["W1"], np.float32))
    W2 = np.ascontiguousarray(np.asarray(inputs["W2"], np.float32))
    Wh1 = np.ascontiguousarray(np.asarray(inputs["Wh1"], np.float32))
    Wh2 = np.ascontiguousarray(np.asarray(inputs["Wh2"], np.float32))
    b0 = np.asarray(inputs["b0"], np.float32)
    b1 = np.asarray(inputs["b1"], np.float32)
    b2 = np.asarray(inputs["b2"], np.float32)
    bh1 = np.asarray(inputs["bh1"], np.float32).reshape(1, P)
    bh2 = np.asarray(inputs["bh2"], np.float32).reshape(1, 2)
    common = dict(
        x=x, w0=W0, w1=W1, w2=W2, wh1=Wh1, wh2=Wh2,
        b0b=np.ascontiguousarray(np.tile(b0.reshape(1, D), (P, 1))),
        b1r=b1.reshape(1, D).copy(),
        bcat=bcat_T(b0, b1, b2),
        bh1=bh1.copy(), bh2=bh2.copy(),
        ident=np.eye(P, dtype=np.float32),
    )
    in_maps = []
    for c in range(N_CORES):
        m = dict(common)
        m["smat"] = pre["s_core"][c]
        m["idx1"] = pre["idx1"][c]
        m["idx2"] = pre["idx2"][c]
        m["uw"] = np.ascontiguousarray(
            pre["uw_core"][c].reshape(P, TPC * 2)
        )
        in_maps.append(m)
    return in_maps


last_results = None  # BassKernelResults of the most recent run (for test.py)


def _build_and_run(inputs, pre):
    import os
    from concourse import bass_utils

    global last_results
    n_nodes = int(inputs["n_nodes"])
    nc = build_nc(pre, n_nodes, pre["sigma_w"])
    in_maps = make_in_maps(inputs, pre)
    trace = os.environ.get("KERNEL_TRACE", "0") == "1"
    res = bass_utils.run_bass_kernel_spmd(
        nc, in_maps, core_ids=list(range(N_CORES)), trace=trace
    )
    last_results = res
    return np.asarray(res.results[0]["out"], np.float32).reshape(2)


def kernel(**inputs):
    n_nodes = int(inputs["n_nodes"])
    pre = preprocess(inputs["edge_index"], n_nodes)
    return _build_and_run(inputs, pre)


def benchmark(inputs, iters=5, repeat=1, stage="full"):
    """Compile once, then time device execution with device-resident inputs.

    Returns (out, per-iter seconds list). Mirrors
    bass2jax.run_bass_via_pjrt's multi-core path with the execution loop
    exposed for timing.
    """
    import time
    import jax
    from jax.sharding import Mesh, PartitionSpec, NamedSharding
    from jax.experimental.shard_map import shard_map
    import concourse.mybir as mybir
    from concourse import bass2jax

    n_nodes = int(inputs["n_nodes"])
    pre = preprocess(inputs["edge_index"], n_nodes)
    nc = build_nc(pre, n_nodes, pre["sigma_w"], repeat=repeat, stage=stage)
    in_maps = make_in_maps(inputs, pre)

    bass2jax.install_neuronx_cc_hook()
    partition_name = (
        nc.partition_id_tensor.name if nc.partition_id_tensor else None
    )
    in_names, out_names, out_avals, zero_outs = [], [], [], []
    for alloc in nc.m.functions[0].allocations:
        if not isinstance(alloc, mybir.MemoryLocationSet):
            continue
        name = alloc.memorylocations[0].name
        if alloc.kind == "ExternalInput":
            if name != partition_name:
                in_names.append(name)
        elif alloc.kind == "ExternalOutput":
            shape = tuple(alloc.tensor_shape)
            dtype = mybir.dt.np(alloc.dtype)
            out_names.append(name)
            out_avals.append(jax.core.ShapedArray(shape, dtype))
            zero_outs.append(np.zeros(shape, dtype))
    n_params = len(in_names)
    all_names = in_names + out_names
    if partition_name is not None:
        all_names = all_names + [partition_name]

    def _body(*args):
        operands = list(args)
        if partition_name is not None:
            operands.append(bass2jax.partition_id_tensor())
        outs = bass2jax._bass_exec_p.bind(
            *operands,
            out_avals=tuple(out_avals),
            in_names=tuple(all_names),
            out_names=tuple(out_names),
            lowering_input_output_aliases=(),
            sim_require_finite=True,
            sim_require_nnan=True,
            nc=nc,
        )
        return tuple(outs)

    devices = jax.devices()[:N_CORES]
    mesh = Mesh(np.asarray(devices), ("core",))
    spec = PartitionSpec("core")
    n_outs = len(out_avals)
    donate = tuple(range(n_params, n_params + n_outs))
    sharded = jax.jit(
        shard_map(
            _body, mesh=mesh, in_specs=(spec,) * (n_params + n_outs),
            out_specs=(spec,) * n_outs, check_rep=False,
        ),
        donate_argnums=donate,
        keep_unused=True,
    )
    sh = NamedSharding(mesh, spec)
    dev_in = [
        jax.device_put(
            np.concatenate(
                [np.asarray(in_maps[c][nm]) for c in range(N_CORES)], axis=0
            ),
            sh,
        )
        for nm in in_names
    ]

    def make_zeros():
        return [
            jax.device_put(
                np.zeros((N_CORES * z.shape[0], *z.shape[1:]), z.dtype), sh
            )
            for z in zero_outs
        ]

    # compile + warmup
    t0 = time.time()
    outs = sharded(*dev_in, *make_zeros())
    jax.block_until_ready(outs)
    compile_s = time.time() - t0

    times = []
    for _ in range(iters):
        zs = make_zeros()
        jax.block_until_ready(zs)
        t0 = time.perf_counter()
        outs = sharded(*dev_in, *zs)
        jax.block_until_ready(outs)
        times.append(time.perf_counter() - t0)

    out0 = np.asarray(outs[out_names.index("out")]).reshape(N_CORES, 1, 2)[0]
    return out0.reshape(2), times, compile_s


if __name__ == "__main__":
    pass

